# revision 6
# baseline (speedup 1.0000x reference)
"""Trainium2 Bass kernel for KernelizedSupCon loss (B=2048, V=2, D=512, 8 cores).

Strategy (data-parallel over anchor rows, per sharding hint):
  - N = B*V = 4096 anchor rows; core c owns rows [c*512, (c+1)*512).
  - Host precomputes, per core: transposed/rotated features fT [D=512, N=4096]
    (so both matmul operands come from one SBUF-resident tensor), the
    delta-count matrix and positive-mask row-blocks (rotated so the diagonal
    block is always column-tile 0 -> one uniform SPMD program), and the
    positive-mask row sums P.
  - Device per core: sim = fT_loc.T @ fT (float32r matmuls, PSUM fp32),
    E = exp((sim-1)/T) on ScalarE, then two fused multiply-reduce passes on
    VectorE: U_n = sum(E * delta), S_n = sum(sim * pm).
  - Host: loss_i = log(U_i) + (1 - S_i/P_i)/T, mean over all rows.

The row-max subtraction in the reference cancels analytically; the constant
shift 1/T (sim_ii ~ 1) keeps exp() in range. delta reduces from the NxN rank
count to V * c_small[i%B, j%B] where c_small[a,b] = #{k: mask[b,k] < mask[b,a]},
computed exactly on host with sort+searchsorted over the fp32 mask (same
comparison semantics as the reference).
"""
import math

import numpy as np

import concourse.bass as bass
import concourse.mybir as mybir
from concourse import tile
from concourse.tile import ScopedClock
from concourse.bass_utils import run_bass_kernel_spmd

TEMPERATURE = 0.07
KRNL_SIGMA = 1.0
B, V, D = 2048, 2, 512
N = B * V
NCORES = 8
R = N // NCORES          # 512 anchor rows per core
MT = R // 128            # 4 partition tiles of local rows
NT = N // 512            # 8 column tiles
KT = D // 128            # 4 contraction tiles
CW = 2560                # stored mask columns: diag patch 512 + slice 2048

_F32 = mybir.dt.float32
_F32R = mybir.dt.float32r


def _patch_tile_drain():
    """Split the Tile tail-drain's sem waits across sync nops (this walrus
    build rejects >2 sync waits on one CTRL instruction)."""
    if getattr(tile.TileContext, "_ant_drain_patched", False):
        return

    def _drain_and_barrier(self, tick_clock, wait_clock):
        nc = self.nc
        collector = nc.sync.nop(nofuse=True)
        wait_clock.add_sem_waits(
            collector.ins, ScopedClock({None: tick_clock.global_clock})
        )
        si = collector.ins.sync_info
        waits = list(si.on_wait) if si and si.on_wait else []
        if si and waits:
            si.on_wait = waits[:1]
        for w in waits[1:]:
            n = nc.sync.nop(nofuse=True)
            n.ins.sync_info = mybir.SyncInfo(on_wait=[w], on_update=[])
        nc.sync.drain()
        nc.all_engine_barrier()
        assert self.sems is not None
        popped = nc._tile_sem_poison_stack.pop()
        assert popped is self._sem_poison
        nc.clear_and_free_semaphores(list(self.sems.allocated().values()))
        nc.all_engine_barrier()

    tile.TileContext._drain_and_barrier = _drain_and_barrier
    tile.TileContext._ant_drain_patched = True


# ---------------------------------------------------------------- host prep

def _mask_small(labels: np.ndarray) -> np.ndarray:
    x = labels.reshape(-1, 1).astype(np.float32)
    d = x - x.T
    return (np.exp(-(d * d) / np.float32(2.0 * KRNL_SIGMA**2)) /
            np.float32(math.sqrt(2.0 * math.pi) * KRNL_SIGMA)).astype(np.float32)


def _c_small(msk: np.ndarray) -> np.ndarray:
    """c[a,b] = #{k : msk[b,k] < msk[b,a]} (strict, fp32 tie semantics)."""
    out = np.empty(msk.shape, dtype=np.float32)
    srt = np.sort(msk, axis=1)
    for b in range(msk.shape[0]):
        out[:, b] = np.searchsorted(srt[b], msk[b], side="left")
    return out


def host_prep(features: np.ndarray, labels: np.ndarray):
    feats = np.transpose(features, (1, 0, 2)).reshape(N, D).astype(np.float32)
    msk = _mask_small(labels)
    delta_small = (np.float32(V) * _c_small(msk)).astype(np.float32)

    in_maps = []
    P_all = np.empty((NCORES, R), dtype=np.float32)
    for c in range(NCORES):
        rows = np.arange(c * R, (c + 1) * R)
        a_idx = rows % B
        jglob = (np.arange(N) + c * R) % N          # rotated column order
        jb = jglob % B

        fT = np.ascontiguousarray(feats[jglob].T)   # [D, N] fp32

        d_loc = delta_small[np.ix_(a_idx, jb)]      # [R, N]
        p_loc = msk[np.ix_(a_idx, jb)]
        rr = np.arange(R)
        d_loc[rr, rr] = 0.0                         # diagonal is rotated col j'=r
        p_loc[rr, rr] = 0.0
        P_all[c] = p_loc.sum(axis=1, dtype=np.float32)

        in_maps.append({
            "fT": fT,
            "dcomb": np.ascontiguousarray(d_loc[:, :CW]),
            "pcomb": np.ascontiguousarray(p_loc[:, :CW]),
        })
    return in_maps, P_all


# ------------------------------------------------------------- device build

def _split_sync_waits(nc: bass.Bass, limit: int = 1):
    """This walrus build rejects instructions carrying more than `limit` sem
    waits; move the overflow onto preceding same-engine nops (engines run in
    program order, so waiting on an earlier nop is equivalent)."""
    import bass_rust
    uid = [0]
    for f in nc.m.functions:
        for bb in f.blocks:
            new_list = []
            for inst in bb.instructions:
                si = inst.sync_info
                waits = list(si.on_wait) if si and si.on_wait else []
                if len(waits) > limit:
                    for i in range(0, len(waits) - limit, limit):
                        chunk = waits[i:i + limit]
                        nop = bass_rust.InstNoOp(
                            name=f"I-waitsplit-{uid[0]}", engine=inst.engine)
                        uid[0] += 1
                        nop.sync_info = mybir.SyncInfo(
                            on_wait=chunk, on_update=[])
                        nc.register_instruction(nop)
                        new_list.append(nop)
                    si.on_wait = waits[len(waits) - limit:]
                new_list.append(inst)
            bb.instructions[:] = new_list


def build_nc() -> bass.Bass:
    _patch_tile_drain()
    nc = bass.Bass("TRN2", target_bir_lowering=False, debug=False,
                   num_devices=NCORES)
    fT_d = nc.dram_tensor("fT", [D, N], _F32R, kind="ExternalInput")
    d_d = nc.dram_tensor("dcomb", [R, CW], _F32, kind="ExternalInput")
    p_d = nc.dram_tensor("pcomb", [R, CW], _F32, kind="ExternalInput")
    u_d = nc.dram_tensor("Uparts", [MT, 128, NT], _F32, kind="ExternalOutput")
    s_d = nc.dram_tensor("Sparts", [MT, 128, NT], _F32, kind="ExternalOutput")

    inv_t = float(1.0 / TEMPERATURE)

    with tile.TileContext(nc) as tc:
        with (
            tc.tile_pool(name="feat", bufs=1) as feat_pool,
            tc.tile_pool(name="masks", bufs=1) as mask_pool,
            tc.tile_pool(name="work", bufs=3) as work_pool,
            tc.tile_pool(name="acc", bufs=2) as acc_pool,
            tc.tile_pool(name="psum", bufs=4, space="PSUM") as psum_pool,
        ):
            bias_t = feat_pool.tile([128, 1], _F32, name="bias")
            nc.vector.memset(bias_t[:], -inv_t)
            fT_t = []
            for k in range(KT):
                ft = feat_pool.tile([128, N], _F32R, name=f"ft{k}")
                nc.sync.dma_start(ft[:], fT_d[k * 128:(k + 1) * 128, :])
                fT_t.append(ft)
            d_t, p_t = [], []
            for m in range(MT):
                dt_ = mask_pool.tile([128, CW], _F32, name=f"dt{m}")
                nc.sync.dma_start(dt_[:], d_d[m * 128:(m + 1) * 128, :])
                d_t.append(dt_)
                pt_ = mask_pool.tile([128, CW], _F32, name=f"pt{m}")
                nc.sync.dma_start(pt_[:], p_d[m * 128:(m + 1) * 128, :])
                p_t.append(pt_)

            for m in range(MT):
                up = acc_pool.tile([128, NT], _F32, name=f"up{m}")
                sp = acc_pool.tile([128, NT], _F32, name=f"sp{m}")
                for n in range(NT):
                    ps = psum_pool.tile([128, 512], _F32, name="ps")
                    for k in range(KT):
                        nc.tensor.matmul(
                            ps[:],
                            lhsT=fT_t[k][:, m * 128:(m + 1) * 128],
                            rhs=fT_t[k][:, n * 512:(n + 1) * 512],
                            start=(k == 0),
                            stop=(k == KT - 1),
                        )
                    e = work_pool.tile([128, 512], _F32, name="e")
                    nc.scalar.activation(
                        e[:], ps[:], mybir.ActivationFunctionType.Exp,
                        bias=bias_t[:], scale=inv_t,
                    )
                    col0 = n * 512 if n <= 4 else (n - 4) * 512
                    scr = work_pool.tile([128, 512], _F32, name="scr")
                    nc.vector.scalar_tensor_tensor(
                        out=scr[:], in0=e[:], scalar=1.0,
                        in1=d_t[m][:, col0:col0 + 512],
                        op0=mybir.AluOpType.mult, op1=mybir.AluOpType.mult,
                        accum_out=up[:, n:n + 1],
                    )
                    scr2 = work_pool.tile([128, 512], _F32, name="scr2")
                    nc.vector.scalar_tensor_tensor(
                        out=scr2[:], in0=ps[:], scalar=1.0,
                        in1=p_t[m][:, col0:col0 + 512],
                        op0=mybir.AluOpType.mult, op1=mybir.AluOpType.mult,
                        accum_out=sp[:, n:n + 1],
                    )
                nc.sync.dma_start(u_d[m], up[:])
                nc.sync.dma_start(s_d[m], sp[:])
    _split_sync_waits(nc)
    return nc


# ------------------------------------------------------------------- kernel

def _postprocess(results, P_all: np.ndarray) -> np.ndarray:
    loss_rows = []
    for c in range(NCORES):
        U = results[c]["Uparts"].reshape(R, NT).sum(axis=1, dtype=np.float32)
        S = results[c]["Sparts"].reshape(R, NT).sum(axis=1, dtype=np.float32)
        P = P_all[c]
        loss_rows.append(np.log(U) + (np.float32(1.0) - S / P) / np.float32(TEMPERATURE))
    loss = np.concatenate(loss_rows).astype(np.float32)
    return np.float32(loss.mean())


def kernel(features: np.ndarray, labels: np.ndarray) -> np.ndarray:
    features = np.asarray(features, dtype=np.float32)
    labels = np.asarray(labels, dtype=np.float32)
    in_maps, P_all = host_prep(features, labels)
    nc = build_nc()
    res = run_bass_kernel_spmd(nc, in_maps, list(range(NCORES)))
    return np.asarray(_postprocess(res.results, P_all), dtype=np.float32)
